# revision 79
# baseline (speedup 1.0000x reference)
"""Trainium2 Bass kernel for nn_Attention_34840774705279 (sparse/deformable attention).

Math (matches reference.py):
  v   = x @ v_w.T + v_b            -> per-head maps [B*NH, H, W, HD]
  off = x @ off_w.T + off_b        -> off_w is structurally zero, so offsets are
                                      CONSTANT per (head, point) => integer shifts.
  w   = softmax_p(x @ aw_w.T + aw_b)
  out[i,j] = sum_p w_p[i,j] * v[i+dy_p, j+dx_p]   (zero outside the map)
  y   = out @ proj_w.T + proj_b

Sharding (8 cores, uniform SPMD program):
  core d -> batch b = d//2, row-half r0 = 64*(d%2). Each core computes ALL 8
  heads for its 64 output rows (8192 tokens) using a 4-row halo of v rows
  (host zero-pads x rows outside the image); host concatenates the results.

Device pipeline (per core), bf16 data path with f32 PSUM accumulation:
  A. v+logit projection, pixel-major: per image row r, a[j, 0:288] =
     xT_row.T @ [v_w.T | aw_w.T] lands in one PSUM bank and is evacuated
     (alternating ScalarE/Pool) into the d-major VL tile [j, 9, 32, 72]
     (slots 0-7 = per-head v maps as [d, row], slot 8 = attention logits).
  B. softmax over the 4 points for all 8 heads (exp on ScalarE, adds +
     reciprocal on VectorE), per row-half.
  C. sampling via weight-then-shift identity
        w .* (S_dx @ V_win) == S_dx @ ((S_-dx^T w) .* V_win):
     per head: 4 tiny matmuls compute column-shifted weights for all points
     into one PSUM bank, one Pool copy evacuates them; per point VectorE
     multiplies the 32-row V window (row shift dy = compile-time slot offset,
     d-major so the weight broadcast is a middle-dim stride-0 and the op runs
     in the 2x DVE mode) and the 0/1 column-shift matrix S_dx matmul-
     accumulates all 4 points in PSUM. o_ps is evacuated to the bf16 OUT
     tile (ScalarE/Pool).
  D. output projection: PE transposes OUT back to channel-major, ScalarE/Pool
     evacuate, y^T = proj.T @ OUT^T accumulates both 128-channel halves in
     PSUM, and the f32 result is DMAd straight from PSUM to DRAM.
  Emission interleaves C-half0 into phase A's tail and D-half0 into C-half1
  so every engine has independent work queued at all times.
"""

import os
import sys
import math

import numpy as np

sys.path.insert(0, "/opt/trn_rl_repo")

P = 128
H = W = 128
NH, NP, HD = 8, 4, 32
DIM = 256
N_TOK = H * W
ROWS_OUT = 64          # output rows per core
HALO = 4
ROWS_V = ROWS_OUT + 2 * HALO   # 72 v-row slots per core
TOK_V = ROWS_V * W             # 9216
N_CORES = 8

DT_MODE = os.environ.get("KERNEL_DT", "bf16")   # "bf16" | "f32r"

_cache = {}


def _build_terms(off_b):
    """Per (h, p): list of (dx, dy, alpha) corner terms from the constant offsets."""
    ob = np.asarray(off_b, np.float64).reshape(NH, NP, 2)
    terms = [[[] for _ in range(NP)] for _ in range(NH)]
    for h in range(NH):
        for p in range(NP):
            fx, fy = ob[h, p, 0], ob[h, p, 1]
            x0 = math.floor(fx)
            y0 = math.floor(fy)
            wx1 = fx - x0
            wy1 = fy - y0
            for dxc, wx in ((x0, 1.0 - wx1), (x0 + 1, wx1)):
                if abs(wx) < 1e-9:
                    continue
                for dyc, wy in ((y0, 1.0 - wy1), (y0 + 1, wy1)):
                    if abs(wy) < 1e-9:
                        continue
                    if abs(dxc) >= W or abs(dyc) > HALO:
                        continue  # fully out of range / beyond halo
                    terms[h][p].append((int(dxc), int(dyc), float(wx * wy)))
    return terms


def _build_smats(terms):
    """Dedupe (dx, alpha) -> [128,128] shift matrices; rewrite terms to
    (s_fwd, s_bwd, dy): out += S_dx @ (V_window * (alpha*S_-dx^T E))."""
    key_to_idx = {}
    mats = []

    def smat(dx, alpha):
        key = (dx, round(alpha, 9))
        if key not in key_to_idx:
            m = np.zeros((P, P), np.float32)
            for j_out in range(W):
                j_in = j_out + dx
                if 0 <= j_in < W:
                    m[j_in, j_out] = alpha
            key_to_idx[key] = len(mats)
            mats.append(m)
        return key_to_idx[key]

    terms2 = [[[] for _ in range(NP)] for _ in range(NH)]
    for h in range(NH):
        for p in range(NP):
            for dx, dy, alpha in terms[h][p]:
                terms2[h][p].append(
                    (smat(dx, 1.0), smat(-dx, alpha), dy))
    return np.stack(mats, 0), terms2


def _np_reference(x, v_w, v_b, aw_w, aw_b, off_w, off_b, proj_w, proj_b, Hh, Ww):
    """Pure-numpy fallback mirroring reference.py (used only if off_w != 0,
    which cannot happen with this problem's setup_inputs)."""
    B, N, C = x.shape
    v = (x @ v_w.T + v_b).reshape(B, N, NH, HD).transpose(0, 2, 1, 3)
    v = v.reshape(B * NH, Hh, Ww, HD)
    mh, mw = np.meshgrid(np.arange(Hh, dtype=x.dtype), np.arange(Ww, dtype=x.dtype),
                         indexing="ij")
    ref = np.stack([mw, mh], -1).reshape(1, N, 1, 2)
    off = (x @ off_w.T + off_b).reshape(B, N, NH, NP, 2).transpose(0, 2, 1, 3, 4)
    off = off.reshape(B * NH, N, NP, 2)
    grid = ref + off
    w = (x @ aw_w.T + aw_b).reshape(B, N, NH, NP).transpose(0, 2, 1, 3)
    w = w.reshape(B * NH, N, NP)
    w = np.exp(w - w.max(-1, keepdims=True))
    w = w / w.sum(-1, keepdims=True)
    G = B * NH
    vf = v.reshape(G, Hh * Ww, HD)
    gx, gy = grid[..., 0], grid[..., 1]
    x0 = np.floor(gx); y0 = np.floor(gy)
    wx1 = gx - x0; wx0 = 1.0 - wx1
    wy1 = gy - y0; wy0 = 1.0 - wy1
    x0i = x0.astype(np.int64); y0i = y0.astype(np.int64)

    def gather(xi, yi):
        valid = (xi >= 0) & (xi < Ww) & (yi >= 0) & (yi < Hh)
        idx = (np.clip(yi, 0, Hh - 1) * Ww + np.clip(xi, 0, Ww - 1))
        g = np.take_along_axis(vf, idx.reshape(G, -1, 1), axis=1)
        return g.reshape(*xi.shape, HD) * valid[..., None]

    samp = ((wy0 * wx0)[..., None] * gather(x0i, y0i)
            + (wy0 * wx1)[..., None] * gather(x0i + 1, y0i)
            + (wy1 * wx0)[..., None] * gather(x0i, y0i + 1)
            + (wy1 * wx1)[..., None] * gather(x0i + 1, y0i + 1))
    out = np.einsum("gnpd,gnp->gnd", samp, w)
    out = out.reshape(B, NH, N, HD).transpose(0, 2, 1, 3).reshape(B, N, C)
    return (out @ proj_w.T + proj_b).astype(np.float32)


def _build_program(terms, n_smats, has_bias=True):
    import concourse.bass as bass
    import concourse.mybir as mybir
    import concourse.tile as tile
    from concourse import bacc

    dt = mybir.dt
    f32 = dt.float32

    fr = dt.bfloat16 if DT_MODE == "bf16" else dt.float32r

    nc = bacc.Bacc("TRN2", target_bir_lowering=False, debug=False,
                   num_devices=N_CORES)

    NCH = 256 + NH * NP  # 288: v channels + aw logits per row matmul

    # ---- DRAM I/O ----
    xt_d = nc.dram_tensor("xt_dev", [DIM, TOK_V], fr, kind="ExternalInput")
    ones_d = nc.dram_tensor("ones_dev", [1, TOK_V], fr, kind="ExternalInput")
    wb_d = nc.dram_tensor("wb_cat", [2, P, NCH], fr, kind="ExternalInput")
    boot_d = nc.dram_tensor("boot", [P, 2 * NCH + 512], fr, kind="ExternalInput")
    bb_d = nc.dram_tensor("bb_cat", [1, NCH], fr, kind="ExternalInput")
    s_d = nc.dram_tensor("s_mats", [P, n_smats, P], fr, kind="ExternalInput")
    pj_d = nc.dram_tensor("proj_t", [P, 2, 2, P], fr, kind="ExternalInput")
    id_d = nc.dram_tensor("ident", [P, P], fr, kind="ExternalInput")
    y_d = nc.dram_tensor("y", [2, P, ROWS_OUT * W], fr, kind="ExternalOutput")

    NG = ROWS_V // 4           # 18 x-DMA groups of 4 rows
    NGO = ROWS_OUT // 4        # 16 groups for phase D

    with tile.TileContext(nc) as tc:
        with (
            tc.tile_pool(name="const", bufs=1) as cpool,
            tc.tile_pool(name="big", bufs=1) as bigpool,
        ):
            abc_pools = (
                tc.tile_pool(name="stA", bufs=2),
                tc.tile_pool(name="psA", bufs=2, space="PSUM"),
                tc.tile_pool(name="psC", bufs=2, space="PSUM"),
                tc.tile_pool(name="wt", bufs=1),
                tc.tile_pool(name="stB", bufs=2),
            )
            stA = abc_pools[0].__enter__()
            psA = abc_pools[1].__enter__()
            psC = abc_pools[2].__enter__()
            wtpool = abc_pools[3].__enter__()
            stB = abc_pools[4].__enter__()

            bb_sb = cpool.tile([1, NCH], fr, tag="bb") if has_bias else None

            # ---- persistent big tiles ----
            # VL: [col j, head(8)+logit(1), d, v-row slot]  (d-major v maps)
            vl_sb = bigpool.tile([P, NH + 1, HD, ROWS_V], fr, tag="V")
            v_sb = vl_sb[:, :NH]
            # OUT: [col j, mh, hl, d, i]  (d-major per-head outputs)
            outs = [bigpool.tile([P, 2, 4, HD, 32], fr, tag="OUT", name="out0"),
                    bigpool.tile([P, 2, 4, HD, 32], fr, tag="OUT2", name="out1")]
            es = [bigpool.tile([P, NH * NP, 32], fr, tag="E", name="e0"),
                  bigpool.tile([P, NH * NP, 32], fr, tag="E2", name="e1")]

            def phase_a_loads(g):
                """Load token groups [g, g+1] (1024 tokens) in one DMA pair.
                Only called for even g; odd groups reuse the same tiles."""
                tok0 = g * 512
                ntok = min(1024, TOK_V - tok0)
                xt_g = [stA.tile([P, 1024], fr, tag=f"xt{kc}", bufs=4,
                                 name=f"xtg{kc}") for kc in range(2)]
                for kc in range(2):
                    nc.sync.dma_start(
                        xt_g[kc][:, :ntok],
                        xt_d[P * kc:P * kc + P, tok0:tok0 + ntok])
                ones_g = None
                if has_bias:
                    ones_g = stA.tile([1, 1024], fr, tag="ones")
                    nc.sync.dma_start(ones_g[:, :ntok],
                                      ones_d[:, tok0:tok0 + ntok])
                return xt_g, ones_g

            lg_state = {}

            def phase_a_mms(g, loaded):
                xt_g, ones_g = loaded
                base = 512 * (g % 2)

                def xsrc(rl, kc, lo, hi):
                    if g == 0 and rl < 2:
                        b0 = 2 * NCH + 256 * kc
                        return boot_sb[:, b0 + lo - base:b0 + hi - base]
                    return xt_g[kc][:, lo:hi]

                if g < 9 and not has_bias:
                    # Pure-A region: two rows of v (2x256) fill one PSUM
                    # bank exactly -> 2 evacuations per group instead of 4
                    # (fixes the 2-engine evacuation bound); the second bank
                    # of each group comes from the idle oacc tag so the bank
                    # rotation also stays under PE's pace. Logits batch 16
                    # rows into the ep bank, one evacuation per 4 groups.
                    if g % 4 == 0:
                        lg_state["t"] = psC.tile([P, 16, 32], f32, tag="ep",
                                                 name="lgb", bufs=1)
                        lg_state["base"] = g
                    lb = lg_state["t"]
                    for pair in range(2):
                        if pair == 0:
                            vb = psA.tile([P, 2, 256], f32, tag="a_ps",
                                          name="a_ps", bufs=3)
                            vba = vb[:]
                        else:
                            vo = psC.tile([P, HD, 32], f32, tag="oacc",
                                          name="a_oacc", bufs=2)
                            vba = vo[:, :16, :].rearrange(
                                "j d i -> j (d i)").rearrange(
                                "j (r c) -> j r c", r=2)
                        for rl2 in range(2):
                            rl = 2 * pair + rl2
                            lo = base + P * rl
                            for kc in range(2):
                                nc.tensor.matmul(
                                    vba[:, rl2, :],
                                    xsrc(rl, kc, lo, lo + P),
                                    wb_sb[:, kc, :256], start=(kc == 0),
                                    stop=(kc == 1))
                                nc.tensor.matmul(
                                    lb[:, 4 * (g - lg_state["base"]) + rl, :],
                                    xsrc(rl, kc, lo, lo + P),
                                    wb_sb[:, kc, 256:], start=(kc == 0),
                                    stop=(kc == 1))
                        rr0 = 4 * g + 2 * pair
                        dst = vl_sb[:, :NH, :, rr0:rr0 + 2]\
                            .rearrange("j h d r -> j r (h d)")
                        eng = (nc.scalar.copy if pair == 0
                               else nc.vector.tensor_copy)
                        eng(dst, vba)
                    if g % 4 == 3 or g == 8:
                        nrows = 4 * (g - lg_state["base"]) + 4
                        r0 = 4 * lg_state["base"]
                        nc.scalar.copy(
                            vl_sb[:, NH, :, r0:r0 + nrows],
                            lb[:, :nrows, :].rearrange("j r p -> j p r"))
                    return
                for rl in range(4):
                    rr = 4 * g + rl      # v-row slot
                    a_tile = psA.tile([P, 512], f32, tag="a_ps",
                                      name="a_ps", bufs=3)
                    a_ps = a_tile[:]
                    for kc in range(2):
                        nc.tensor.matmul(
                            a_ps[:, :NCH],
                            xsrc(rl, kc, base + P * rl, base + P * rl + P),
                            wb_sb[:, kc, :], start=(kc == 0),
                            stop=(kc == 1 and not has_bias))
                    if has_bias:
                        nc.tensor.matmul(
                            a_ps[:, :NCH],
                            ones_g[:, base + P * rl:base + P * rl + P],
                            bb_sb[:], start=False, stop=True)
                    dst = vl_sb[:, :, :, rr].rearrange("j h d -> j (h d)")
                    eng = (nc.scalar.copy if rr % 2 == 0
                           else nc.vector.tensor_copy)
                    eng(dst, a_ps[:, :NCH])

            def phase_a(g):
                """x rows 4g..4g+4: v-proj + logits, pixel-major."""
                phase_a_mms(g, phase_a_loads(g))

            def load_consts():
                s_sb = cpool.tile([P, n_smats, P], fr, tag="smats")
                nc.sync.dma_start(s_sb[:], s_d[:])
                pj_sb = cpool.tile([P, 2, 2, P], fr, tag="proj")
                nc.sync.dma_start(pj_sb[:], pj_d[:])
                id_sb = cpool.tile([P, P], fr, tag="ident")
                nc.sync.dma_start(id_sb[:], id_d[:])
                return s_sb, pj_sb, id_sb

            def phase_b_exp(half, rows):
                """exp of the attention logits for a row slice of `half` —
                emitted as soon as those logit rows are evacuated so only the
                short normalize chain remains at the phase transition."""
                lo, hi = rows
                rr = 32 * half
                e_sb = es[half]
                nc.scalar.activation(
                    e_sb[:, :, lo:hi],
                    vl_sb[:, NH, :, HALO + rr + lo:HALO + rr + hi],
                    mybir.ActivationFunctionType.Exp)

            def phase_b(half, hgrp=None):
                """exp + softmax over points; hgrp=(h0,h1) limits to a head
                range so the chain can straddle other work."""
                h0, h1 = (0, NH) if hgrp is None else hgrp
                rr = 32 * half
                e_sb = es[half]
                nc.scalar.activation(
                    e_sb[:, NP * h0:NP * h1, :],
                    vl_sb[:, NH, NP * h0:NP * h1, HALO + rr:HALO + rr + 32],
                    mybir.ActivationFunctionType.Exp)
                z = stB.tile([P, NH, 32], f32, tag="z", name=f"z{half}_{h0}")
                zr = stB.tile([P, NH, 32], fr, tag="zr", name=f"zr{half}_{h0}")
                ev = e_sb[:].rearrange("j (h p) i -> j h p i", p=NP)
                nc.gpsimd.tensor_tensor(z[:, h0:h1], ev[:, h0:h1, 0, :],
                                         ev[:, h0:h1, 1, :],
                                         op=mybir.AluOpType.add)
                nc.gpsimd.tensor_tensor(z[:, h0:h1], z[:, h0:h1],
                                        ev[:, h0:h1, 2, :],
                                        op=mybir.AluOpType.add)
                nc.gpsimd.tensor_tensor(z[:, h0:h1], z[:, h0:h1],
                                        ev[:, h0:h1, 3, :],
                                        op=mybir.AluOpType.add)
                with nc.allow_low_precision(reason="softmax weights tolerate bf16"):
                    nc.vector.reciprocal(zr[:, h0:h1], z[:, h0:h1])
                    nc.vector.tensor_tensor(
                        ev[:, h0:h1], ev[:, h0:h1],
                        zr[:, h0:h1].unsqueeze(2).broadcast_to(
                            [P, h1 - h0, NP, 32]),
                        op=mybir.AluOpType.mult)

            def head_terms(h):
                """Flat [(p, s_fwd, s_bwd, dy), ...] for head h (<=16 fits one
                PSUM bank)."""
                return [(p, s_fwd, s_bwd, dy)
                        for p in range(NP)
                        for (s_fwd, s_bwd, dy) in terms[h][p]]

            def phase_c_ep(half, h4):
                """Stage 1 for heads h4..h4+4: all column-shifted weight
                vectors land in one PSUM bank; a single evacuation serves
                four heads' sampling stages."""
                e_sb = es[half]
                ep_ps = psC.tile([P, 4, NP, 32], f32, tag="ep", bufs=1,
                                 name=f"epps{half}_{h4}")
                for hl in range(4):
                    for t, (p, s_fwd, s_bwd, dy) in enumerate(
                            head_terms(h4 + hl)):
                        nc.tensor.matmul(
                            ep_ps[:, hl, t, :], s_sb[:, s_bwd, :],
                            e_sb[:, NP * (h4 + hl) + p, :],
                            start=True, stop=True)
                ep = wtpool.tile([P, 4, NP, 32], fr, tag="ep_sb", bufs=3,
                                 name=f"ep{half}_{h4}")
                nc.scalar.copy(ep[:], ep_ps[:])
                return ep

            pending_pool = {}

            def emit_pool_mt(half, h, ep4, t):
                """Pool (SBUF-only) weighting multiply for head h's term t."""
                (p, s_fwd, s_bwd, dy) = head_terms(h)[t]
                slot0 = 32 * half + dy + HALO
                m_t = wtpool.tile([P, HD, 32], fr, tag="wtp", bufs=3,
                                  name="mtp")
                with nc.allow_low_precision(reason="bf16 sampling"):
                    nc.gpsimd.tensor_tensor(
                        m_t[:], v_sb[:, h, :, slot0:slot0 + 32],
                        ep4[:, h % 4, t, :].unsqueeze(1).broadcast_to(
                            [P, HD, 32]),
                        op=mybir.AluOpType.mult)
                return m_t

            def phase_c_acc(half, h, ep4, nxt=None):
                """Stage 2 for head h: weight V windows (one term on Pool —
                SBUF-only — pre-emitted a head early so it never gates; the
                rest on VectorE) and shift-accumulate all points in PSUM
                (PE), then evacuate to OUT. `nxt`=(h_next, ep4_next) pre-
                emits the next head's Pool multiply."""
                ep = ep4[:, h % 4]
                rr = 32 * half
                mh, hl = h // 4, h % 4
                o_ps = psC.tile([P, HD, 32], f32, tag="oacc", bufs=2)
                tl = head_terms(h)
                tps = [len(tl) - 1] if len(tl) > 1 else []
                tp_set = set(tps)
                mts = {}
                if tps:
                    key = (half, h)
                    if key in pending_pool:
                        mts[tps[0]] = pending_pool.pop(key)
                    else:
                        mts[tps[0]] = emit_pool_mt(half, h, ep4, tps[0])
                if nxt is not None:
                    hn, ep4n = nxt
                    tln = head_terms(hn)
                    if len(tln) > 1:
                        pending_pool[(half, hn)] = emit_pool_mt(
                            half, hn, ep4n, len(tln) - 1)
                for t in [t for t in range(len(tl)) if t not in tp_set]:
                    m_t = wtpool.tile([P, HD, 32], fr, tag=f"wt{t % 2}",
                                      bufs=3, name=f"mt{t % 2}")
                    (p, s_fwd, s_bwd, dy) = tl[t]
                    slot0 = rr + dy + HALO
                    with nc.allow_low_precision(reason="bf16 sampling"):
                        nc.vector.tensor_tensor(
                            m_t[:], v_sb[:, h, :, slot0:slot0 + 32],
                            ep[:, t, :].unsqueeze(1).broadcast_to(
                                [P, HD, 32]),
                            op=mybir.AluOpType.mult)
                    mts[t] = m_t
                acc_order = [t for t in range(len(tl))
                             if t not in tp_set] + tps[::-1]
                for n, t in enumerate(acc_order):
                    (p, s_fwd, s_bwd, dy) = tl[t]
                    for ch in range(2):
                        nc.tensor.matmul(
                            o_ps[:, 16 * ch:16 * ch + 16, :]
                            .rearrange("j d i -> j (d i)"),
                            s_sb[:, s_fwd, :],
                            mts[t][:, 16 * ch:16 * ch + 16, :]
                            .rearrange("j d i -> j (d i)"),
                            start=(n == 0),
                            stop=(n == len(tl) - 1))
                nc.scalar.copy(outs[half][:, mh, hl], o_ps[:])

            ysbs = [None, None]

            def phase_d_t(halfd, g):
                """transposes + OT evacuations for one row group."""
                i0 = 4 * g - 32 * halfd
                ot_sb = []
                for m in range(2):
                    ot_ps = psA.tile([P, 512], f32, tag="a_ps",
                                     name=f"ot{m}", bufs=3)
                    otv = ot_ps[:].bitcast(fr)
                    for c in range(4):
                        nc.tensor.transpose(
                            otv[:, P * c:P * (c + 1)],
                            outs[halfd][:, m, :, :, i0 + c]
                            .rearrange("j hl d -> j (hl d)"),
                            id_sb[:])
                    t = stA.tile([P, 512], fr, tag=f"ot{m}", bufs=3)
                    nc.vector.tensor_copy(t[:], otv[:, :512])
                    ot_sb.append(t)
                return ot_sb

            def phase_d(halfd, gls=None, pipelined=False):
                """output projection for row groups of half `halfd`. With
                pipelined=True, group g+1's transposes are emitted before
                group g's projection matmuls so PE never waits on the OT
                evacuation chain."""
                gl_list = list(range(NGO // 2) if gls is None else gls)
                if pipelined:
                    pend = None
                    for gl in gl_list:
                        g = halfd * (NGO // 2) + gl
                        ot_sb = phase_d_t(halfd, g)
                        if pend is not None:
                            phase_d_y(halfd, pend[0], pend[1])
                        pend = (g, ot_sb)
                    phase_d_y(halfd, pend[0], pend[1])
                    return
                for gl in gl_list:
                    g = halfd * (NGO // 2) + gl
                    ot_sb = phase_d_t(halfd, g)
                    phase_d_y(halfd, g, ot_sb)

            def phase_d_y(halfd, g, ot_sb):
                    y_pss = []
                    for mc in range(2):
                        if halfd == 1 and mc == 0:
                            y_ps = psC.tile([P, NP * 4, 32], f32, tag="ep",
                                            name="yps0d1", bufs=1
                                            )[:].rearrange("j a b -> j (a b)")
                        else:
                            y_ps = psA.tile([P, 512], f32, tag="a_ps",
                                            name=f"yps{mc}", bufs=3)[:]
                        y_pss.append(y_ps)
                    # kc-major order: the kc=0 matmuls only need ot_sb[0], so
                    # PE starts as soon as the first evacuation lands.
                    for kc in range(2):
                        for mc in range(2):
                            nc.tensor.matmul(y_pss[mc],
                                             pj_sb[:, kc, mc, :],
                                             ot_sb[kc][:],
                                             start=(kc == 0), stop=(kc == 1))
                    ysb = ysbs[0]
                    tail3 = halfd == 1 and g >= NGO - 4
                    if tail3:
                        # last four groups share one wide tile: DMA g12-14 as
                        # one transfer once g14 lands, then a short final g15
                        if g == NGO - 4:
                            ysb = stA.tile([P, 2, 1024], fr, tag="ylast",
                                           name="ysbl", bufs=2)
                            ysbs[0] = ysb
                        elif g % 2 == 0:
                            ysb = stA.tile([P, 2, 1024], fr, tag="ylast",
                                           name="ysbl2", bufs=2)
                            ysbs[0] = ysb
                        else:
                            ysb = ysbs[0]
                        h0 = 512 * (g % 2)
                    elif ysb is None or g % 2 == 0:
                        ysb = stA.tile([P, 2, 1024], fr, tag="y",
                                       name="ysb", bufs=3)
                        ysbs[0] = ysb
                        h0 = 512 * (g % 2)
                    else:
                        h0 = 512 * (g % 2)
                    for mc in range(2):
                        # D1: split y evacuations Act/DVE (DVE has slack
                        # there and Act otherwise serializes the drain)
                        eng = (nc.vector.tensor_copy
                               if halfd == 1 and mc == 1 else nc.scalar.copy)
                        eng(ysb[:, mc, h0:h0 + 512], y_pss[mc])
                    if tail3:
                        nc.sync.dma_start(
                            y_d[:, :, 512 * g:512 * (g + 1)]
                            .rearrange("mc j t -> j mc t"),
                            ysb[:, :, h0:h0 + 512])
                    elif g % 2 == 1:
                        nc.sync.dma_start(
                            y_d[:, :, 512 * (g - 1):512 * (g + 1)]
                            .rearrange("mc j t -> j mc t"),
                            ysb[:])

            # ---- PE warm-up: the cost model ramps PE to full rate only
            # after 3us of continuous execution. Dummy matmuls over a
            # memset tile keep PE busy from ~0.4us so the first real
            # projection matmuls (data-ready ~3.7us) run at full speed.
            NW_BIG = int(os.environ.get("KERNEL_WARM", "3"))
            if NW_BIG:
                warm = cpool.tile([P, 512], fr, tag="warm")
                nc.gpsimd.memset(warm[:], 0)
                wp = psC.tile([P, HD, 32], f32, tag="oacc",
                              name="warmps", bufs=2)
                wps = wp[:, :16, :].rearrange("j d i -> j (d i)")
                for _ in range(NW_BIG):
                    nc.tensor.matmul(wps, warm[:, :P], warm[:],
                                     start=True, stop=True)
                for _ in range(int(os.environ.get("KERNEL_WARMT", "10"))):
                    nc.tensor.matmul(wps[:, :32], warm[:, :P], warm[:, :32],
                                     start=True, stop=True)

            # ---- emission order: small xt chunk, weights, rest ----
            # one boot DMA carries wb + the first two x rows of both
            # contraction halves: the first matmul waits on a single
            # issue+transfer instead of three serialized ones
            boot_sb = cpool.tile([P, 2 * NCH + 512], fr, tag="boot")
            nc.sync.dma_start(boot_sb[:], boot_d[:])
            wb_sb = boot_sb[:, :2 * NCH].rearrange("j (kc f) -> j kc f", kc=2)
            xt01_t = [stA.tile([P, 1024], fr, tag=f"xt{kc}", bufs=4,
                               name=f"xtg{kc}") for kc in range(2)]
            if has_bias:
                nc.sync.dma_start(bb_sb[:], bb_d[:])
            for kc in range(2):
                nc.sync.dma_start(xt01_t[kc][:, 256:],
                                  xt_d[P * kc:P * kc + P, 256:1024])
            ones01 = None
            if has_bias:
                ones01 = stA.tile([1, 1024], fr, tag="ones")
                nc.sync.dma_start(ones01[:], ones_d[:, :1024])
            xt01 = (xt01_t, ones01)
            xt23 = phase_a_loads(2)
            phase_a_mms(0, xt01)
            phase_a_mms(1, xt01)
            xt45 = phase_a_loads(4)
            phase_a_mms(2, xt23)
            phase_a_mms(3, xt23)
            s_sb, pj_sb, id_sb = load_consts()
            cur = xt45
            for g in range(4, 9):
                if g % 2 == 0 and g > 4:
                    cur = phase_a_loads(g)
                phase_a_mms(g, cur)
                if g == 8:
                    phase_b(0, (0, NH // 2))

            PH = os.environ.get("KERNEL_PHASES", "abcd")
            nc0 = NH if "c" in PH else 0
            # C0 interleave starts at g=9: heads h0/h4 (dy=0) only need V
            # slots 4-35 (ready after g8) and their half of B0.
            order0 = [0, 4, 1, 2, 3, 5, 6, 7]
            ci = 0
            ep4s = {}
            for g in range(9, NG):
                if g % 2 == 0:
                    cur = phase_a_loads(g)
                h = order0[ci] if ci < nc0 else None
                if h is not None and (h // 4) not in ep4s:
                    ep4s[h // 4] = phase_c_ep(0, 4 * (h // 4))
                phase_a_mms(g, cur)
                if g == 9:
                    phase_b(0, (NH // 2, NH))
                if "c" in PH and g == NG - 2:
                    phase_b(1, (0, NH // 2))
                elif "c" in PH and g == NG - 1:
                    phase_b(1, (NH // 2, NH))
                    if nc0:
                        ep4s["c1"] = phase_c_ep(1, 0)
                if h is not None:
                    hn = order0[ci + 1] if ci + 1 < nc0 else None
                    nxt = ((hn, ep4s[hn // 4])
                           if hn is not None and hn // 4 in ep4s else None)
                    phase_c_acc(0, h, ep4s[h // 4], nxt=nxt)
                    ci += 1
            for h in range(NH):
                if "c" in PH and h == 4:
                    ep4s["c2"] = phase_c_ep(1, 4)
                if "d" in PH:
                    phase_d(0, gls=[h])
                if "c" in PH:
                    kn = "c1" if h + 1 < 4 else "c2"
                    nxt = ((h + 1, ep4s[kn])
                           if h + 1 < NH and kn in ep4s else None)
                    phase_c_acc(1, h, ep4s["c1" if h < 4 else "c2"], nxt=nxt)
            if "d" in PH:
                phase_d(1, pipelined=True)
            for pl in reversed(abc_pools):
                pl.__exit__(None, None, None)

    nc.compile()
    return nc


def kernel(x, v_w, v_b, aw_w, aw_b, off_w, off_b, proj_w, proj_b, H=128, W=128,
           **_unused):
    x = np.ascontiguousarray(np.asarray(x, np.float32))
    v_w = np.asarray(v_w, np.float32); v_b = np.asarray(v_b, np.float32)
    aw_w = np.asarray(aw_w, np.float32); aw_b = np.asarray(aw_b, np.float32)
    off_w = np.asarray(off_w, np.float32); off_b = np.asarray(off_b, np.float32)
    proj_w = np.asarray(proj_w, np.float32); proj_b = np.asarray(proj_b, np.float32)

    if (np.any(off_w != 0.0) or int(H) != 128 or int(W) != 128
            or np.any(proj_b != 0.0)):
        # data-dependent offsets / non-128 map / proj bias: exact host fallback
        return _np_reference(x, v_w, v_b, aw_w, aw_b, off_w, off_b,
                             proj_w, proj_b, int(H), int(W))

    import ml_dtypes
    np_fr = ml_dtypes.bfloat16 if DT_MODE == "bf16" else np.float32

    terms = _build_terms(off_b)
    s_mats, terms2 = _build_smats(terms)

    has_bias = bool(np.any(v_b) or np.any(aw_b))
    key = ("prog", DT_MODE, s_mats.shape[0], has_bias,
           tuple(tuple(tuple(tl) for tl in th) for th in terms2))
    if key not in _cache:
        _cache[key] = _build_program(terms2, s_mats.shape[0], has_bias)
    nc = _cache[key]

    B = x.shape[0]
    # ---- host prep, shared across cores ----
    NCH = 256 + NH * NP
    wb_cat = np.empty((2, P, NCH), np.float32)
    for kc in range(2):
        wb_cat[kc, :, :256] = v_w[:, P * kc:P * (kc + 1)].T
        wb_cat[kc, :, 256:] = aw_w[:, P * kc:P * (kc + 1)].T
    bb_cat = np.concatenate([v_b, aw_b]).reshape(1, NCH)
    pj_t = np.empty((2, 2, P, P), np.float32)
    for kc in range(2):
        for mc in range(2):
            pj_t[kc, mc] = proj_w[P * mc:P * (mc + 1), P * kc:P * (kc + 1)].T
    ident = np.eye(P, dtype=np.float32)
    shared = dict(wb_cat=np.ascontiguousarray(wb_cat).astype(np_fr),
                  bb_cat=np.ascontiguousarray(bb_cat).astype(np_fr),
                  s_mats=np.ascontiguousarray(
                      s_mats.transpose(1, 0, 2)).astype(np_fr),
                  proj_t=np.ascontiguousarray(
                      pj_t.transpose(2, 0, 1, 3)).astype(np_fr),
                  ident=ident.astype(np_fr))

    xr = x.reshape(B, H, W, DIM)
    in_maps = []
    for d in range(N_CORES):
        b, half = d // 2, d % 2
        r0 = ROWS_OUT * half
        x_dev = np.zeros((ROWS_V, W, DIM), np.float32)
        ones = np.zeros((ROWS_V, W), np.float32)
        lo, hi = max(0, r0 - HALO), min(H, r0 + ROWS_OUT + HALO)
        x_dev[lo - (r0 - HALO):hi - (r0 - HALO)] = xr[b, lo:hi]
        ones[lo - (r0 - HALO):hi - (r0 - HALO)] = 1.0
        m = dict(shared)
        xt_host = np.ascontiguousarray(
            x_dev.reshape(TOK_V, DIM).T).astype(np_fr)
        m["xt_dev"] = xt_host
        boot = np.empty((P, 2 * NCH + 512), np_fr)
        boot[:, :2 * NCH] = (
            m["wb_cat"].transpose(1, 0, 2).reshape(P, 2 * NCH))
        boot[:, 2 * NCH:2 * NCH + 256] = xt_host[:P, :256]
        boot[:, 2 * NCH + 256:] = xt_host[P:, :256]
        m["boot"] = boot
        m["ones_dev"] = ones.reshape(1, TOK_V).astype(np_fr)
        in_maps.append(m)

    from concourse import bass_utils
    res = bass_utils.run_bass_kernel_spmd(
        nc, in_maps, core_ids=list(range(N_CORES)),
        trace=os.environ.get("KERNEL_TRACE", "0") == "1")
    kernel.last_results = res

    y = np.empty((B, N_TOK, DIM), np.float32)
    for d in range(N_CORES):
        b, half = d // 2, d % 2
        yd = np.asarray(res.results[d]["y"], np.float32).reshape(2 * P, -1)
        y[b, ROWS_OUT * W * half:ROWS_OUT * W * (half + 1), :] = yd.T
    return y
